# revision 13
# baseline (speedup 1.0000x reference)
"""Multi-head attention (B=2, S=2048, D=1024, H=16) on 8 Trainium2 NeuronCores.

Sharding: core c -> batch b = c // 4, head-group g = c % 4 (4 heads = 256 proj
dims per core). Each core computes its 4 heads' attention plus the matching
slice of the output projection; the host sums the 4 partial outputs per batch
and adds bo.

Device layouts (matmul operands float32r = fp32 bits at bf16 PE rate):
  qT/kT [o, s]   : proj from host-transposed Q/K (contraction on partitions)
  v     [s, o]   : natural layout + ones column per head (softmax denominator
                   rides along row 64 of the PV matmul output)
  scoresT [k, q] : head pairs row-packed on the PE (base_partition 0/64);
                   both halves of a [128,1024] PSUM tile -> one wide Exp
  outT  [d, q]   : unnormalized; moved off PSUM fast, normalized with
                   reciprocal_approx_fast + GpSimd partition_broadcast
  out_pT [o, q]  : local slice of x @ Wo.T; host transposes + sums + bias
"""

import ml_dtypes
import numpy as np

import concourse.bass as bass
import concourse.mybir as mybir
import concourse.tile as tile
from concourse import bacc
from concourse.bass_utils import run_bass_kernel_spmd

B, S, D, H = 2, 2048, 1024, 16
OL = 256          # local projection dims (4 heads x 64)
NI = D // 128     # contraction chunks for projections
NK = S // 128     # key chunks
NQ = S // 512     # query blocks

_CACHE = {}


def _build():
    DT = mybir.dt.float16
    F32 = mybir.dt.float32
    AF = mybir.ActivationFunctionType

    nc = bacc.Bacc("TRN2", target_bir_lowering=False, debug=False, num_devices=8)

    qt_d = nc.dram_tensor("qt", [D, S], DT, kind="ExternalInput").ap() \
        .rearrange("(c p) s -> c p s", p=128)
    kt_d = nc.dram_tensor("kt", [D, S], DT, kind="ExternalInput").ap() \
        .rearrange("(c p) s -> c p s", p=128)
    vt_d = nc.dram_tensor("vt", [D, S], DT, kind="ExternalInput").ap() \
        .rearrange("(c p) s -> c p s", p=128)
    wq_d = nc.dram_tensor("wqt", [D, OL], DT, kind="ExternalInput").ap() \
        .rearrange("(c p) o -> c p o", p=128)
    wk_d = nc.dram_tensor("wkt", [D, OL], DT, kind="ExternalInput").ap() \
        .rearrange("(c p) o -> c p o", p=128)
    wv_d = nc.dram_tensor("wvt", [D, OL], DT, kind="ExternalInput").ap() \
        .rearrange("(c p) o -> c p o", p=128)
    bq_d = nc.dram_tensor("bq2", [2, 128, 1], F32, kind="ExternalInput").ap()
    bk_d = nc.dram_tensor("bk2", [2, 128, 1], F32, kind="ExternalInput").ap()
    bv_d = nc.dram_tensor("bv1", [1, OL], DT, kind="ExternalInput").ap()
    wo_d = nc.dram_tensor("wot", [OL, D], DT, kind="ExternalInput").ap() \
        .rearrange("(c p) o -> c p o", p=128)
    out_d = nc.dram_tensor("out_t", [D, S], F32, kind="ExternalOutput").ap() \
        .rearrange("(c p) s -> c p s", p=128)

    with tile.TileContext(nc) as tc:
        with (
            tc.tile_pool(name="per", bufs=1) as per,
            tc.tile_pool(name="wp", bufs=1) as wp,
            tc.tile_pool(name="ip", bufs=1) as ip,
            tc.tile_pool(name="pr", bufs=8) as pr,
            tc.tile_pool(name="sm", bufs=2) as sm,
            tc.tile_pool(name="ot", bufs=2) as ot,
            tc.tile_pool(name="osg", bufs=3) as osg,
            tc.tile_pool(name="pj", bufs=2, space="PSUM") as pj,
            tc.tile_pool(name="p1", bufs=2, space="PSUM") as p1,
            tc.tile_pool(name="px", bufs=2, space="PSUM") as px,
        ):
            # --- persistent tiles
            qt_sb = [per.tile([128, S], DT, tag=f"qt{m}", name=f"qt{m}")
                     for m in range(2)]
            kt_sb = [per.tile([128, S], DT, tag=f"kt{m}", name=f"kt{m}")
                     for m in range(2)]
            v_sb = [per.tile([128, 4, 65], DT, tag=f"v{sc}", name=f"v{sc}")
                    for sc in range(NK)]
            wo_sb = [per.tile([128, D], DT, tag=f"wo{c}", name=f"wo{c}")
                     for c in range(2)]
            bq_sb = [per.tile([128, 1], F32, tag=f"bq{m}", name=f"bq{m}")
                     for m in range(2)]
            bk_sb = [per.tile([128, 1], F32, tag=f"bk{m}", name=f"bk{m}")
                     for m in range(2)]
            bv_sb = per.tile([1, OL], DT, tag="bv", name="bv")
            ones_f = per.tile([1, 128], F32, tag="ones_f", name="ones_f")
            vones_f = per.tile([128, 1], F32, tag="vones_f", name="vones_f")
            ones_r = per.tile([1, 128], DT, tag="ones_r", name="ones_r")
            nc.vector.memset(ones_f[:], 1.0)
            nc.vector.memset(vones_f[:], 1.0)
            nc.vector.tensor_copy(ones_r[:], ones_f[:])

            for m in range(2):
                nc.sync.dma_start(bq_sb[m][:], bq_d[m])
                nc.sync.dma_start(bk_sb[m][:], bk_d[m])
            nc.sync.dma_start(bv_sb[:], bv_d)
            for c in range(2):
                nc.sync.dma_start(wo_sb[c][:], wo_d[c])

            # --- input loads on the two HWDGE engines (Sync + Scalar).
            # Column-chunked so chains start as slices land; per-queue DMA
            # is ~25 GB/s, so chunks spread across the 8 physical queues.
            def load_w(w_dr, key, eng):
                ws = []
                for i in range(NI):
                    w = wp.tile([128, OL], DT, tag=f"w{key}{i}",
                                name=f"w{key}{i}")
                    eng.dma_start(w[:], w_dr[i])
                    ws.append(w)
                return ws

            def load_a(a_dr, key, eng, nchunk):
                as_ = [ip.tile([128, S], DT, tag=f"a{key}{i}",
                               name=f"a{key}{i}") for i in range(NI)]
                width = S // nchunk
                for s4 in range(nchunk):
                    csl = slice(s4 * width, (s4 + 1) * width)
                    for i in range(NI):
                        eng.dma_start(as_[i][:, csl], a_dr[i][:, csl])
                return as_

            ws_k = load_w(wk_d, "k", nc.sync)
            ws_q = load_w(wq_d, "q", nc.scalar)
            as_k = load_a(kt_d, "k", nc.sync, 4)
            as_q = load_a(qt_d, "q", nc.scalar, 4)
            ws_v = load_w(wv_d, "v", nc.sync)
            as_v = load_a(vt_d, "v", nc.sync, 2)

            def proj_qk(ws, as_, bias_sb, dst_sb):
                # dst[o, s] = sum_i W[o, i] X[s, i]
                for s in range(4):
                    for m in range(2):
                        acc = pj.tile([128, 512], F32, tag="pj", name="pj")
                        for i in range(NI):
                            nc.tensor.matmul(
                                acc[:],
                                ws[i][:, m * 128:(m + 1) * 128],
                                as_[i][:, s * 512:(s + 1) * 512],
                                start=(i == 0),
                                stop=(i == NI - 1),
                            )
                        nc.scalar.activation(
                            dst_sb[m][:, s * 512:(s + 1) * 512],
                            acc[:], AF.Identity, bias=bias_sb[m][:],
                        )

            def v_chain(sc):
                # one v[s, o] output chunk: 8-deep accumulation + bias matmul
                acc = pj.tile([128, OL], F32, tag="pj", name="pj")
                for i in range(NI):
                    nc.tensor.matmul(
                        acc[:],
                        as_v[i][:, sc * 128:(sc + 1) * 128],
                        ws_v[i][:],
                        start=(i == 0),
                        stop=False,
                    )
                nc.tensor.matmul(
                    acc[:], ones_r[:], bv_sb[:], start=False, stop=True
                )
                for h in range(4):
                    nc.vector.tensor_copy(
                        v_sb[sc][:, h, 0:64],
                        acc[:, h * 64:(h + 1) * 64],
                    )

            for sc in range(NK):
                nc.vector.tensor_copy(
                    v_sb[sc][:, :, 64:65],
                    vones_f[:].to_broadcast((128, 4, 1)),
                )
            proj_qk(ws_k, as_k, bk_sb, kt_sb)
            for sc in range(NK):
                v_chain(sc)
            proj_qk(ws_q, as_q, bq_sb, qt_sb)

            # --- attention + output projection, per query block
            # OP of qb-1 is spread into qb's pair-0 kc loop (PE slack there);
            # PV matmuls trail the exp by 2 kc steps so the PE never waits.
            def emit_op(qb, ots_prev, pools):
                for oc in range(8):
                    osl = slice(oc * 128, (oc + 1) * 128)
                    pool, tg = pools[oc % len(pools)]
                    pso = pool.tile([128, 512], F32, tag=tg, name="pso")
                    for c in range(2):
                        nc.tensor.matmul(
                            pso[:], wo_sb[c][:, osl], ots_prev[c][:],
                            start=(c == 0), stop=(c == 1),
                        )
                    st = osg.tile([128, 512], F32, tag="st", name="st")
                    nc.vector.tensor_copy(st[:], pso[:])
                    nc.sync.dma_start(
                        out_d[oc][:, qb * 512:(qb + 1) * 512], st[:])

            ots_prev = None
            for qb in range(NQ):
                qsl = slice(qb * 512, (qb + 1) * 512)
                ots = [ot.tile([128, 512], DT, tag=f"c{c}", name=f"otc{c}")
                       for c in range(2)]
                for pair in range(2):
                    acc = [px.tile([65, 512], F32, tag="x", name="acc")
                           for _ in range(2)]
                    pend = []
                    op_iter = None
                    if pair == 0 and ots_prev is not None:
                        op_iter = iter(range(8))
                    for kc in range(NK):
                        ksl = slice(kc * 128, (kc + 1) * 128)
                        ps1 = p1.tile([128, 1024], F32, tag="s", name="s")
                        for hh in range(2):
                            psl = slice(hh * 64, (hh + 1) * 64)
                            nc.tensor.matmul(
                                ps1[:, hh * 512:(hh + 1) * 512],
                                kt_sb[pair][psl, ksl],
                                qt_sb[pair][psl, qsl],
                                start=True, stop=True,
                            )
                        prob = pr.tile([128, 1024], DT, tag="p", name="p")
                        nc.scalar.activation(
                            prob[:], ps1[:], AF.Exp, scale=0.125
                        )
                        pend.append((kc, prob))
                        if len(pend) > 2:
                            pkc, pprob = pend.pop(0)
                            for hh in range(2):
                                nc.tensor.matmul(
                                    acc[hh][:], v_sb[pkc][:, pair * 2 + hh, :],
                                    pprob[:, hh * 512:(hh + 1) * 512],
                                    start=(pkc == 0), stop=(pkc == NK - 1),
                                )
                        if op_iter is not None and kc % 2 == 1:
                            oc = next(op_iter, None)
                            if oc is not None:
                                osl = slice(oc * 128, (oc + 1) * 128)
                                pso = pj.tile([128, 512], F32, tag="pj",
                                              name="pso")
                                for c in range(2):
                                    nc.tensor.matmul(
                                        pso[:], wo_sb[c][:, osl],
                                        ots_prev[c][:],
                                        start=(c == 0), stop=(c == 1),
                                    )
                                st = osg.tile([128, 512], F32, tag="st",
                                              name="st")
                                nc.vector.tensor_copy(st[:], pso[:])
                                nc.sync.dma_start(
                                    out_d[oc][:, (qb - 1) * 512:qb * 512],
                                    st[:])
                    for pkc, pprob in pend:
                        for hh in range(2):
                            nc.tensor.matmul(
                                acc[hh][:], v_sb[pkc][:, pair * 2 + hh, :],
                                pprob[:, hh * 512:(hh + 1) * 512],
                                start=(pkc == 0), stop=(pkc == NK - 1),
                            )
                    # normalize off-bank: free both acc banks first
                    uns, dens = [], []
                    for hh in range(2):
                        un = sm.tile([64, 512], F32, tag=f"un{hh}",
                                     name=f"un{hh}")
                        nc.vector.tensor_copy(un[:], acc[hh][0:64, :])
                        den = sm.tile([1, 512], F32, tag=f"den{hh}",
                                      name=f"den{hh}")
                        nc.vector.tensor_copy(den[:], acc[hh][64:65, :])
                        uns.append(un)
                        dens.append(den)
                    for hh in range(2):
                        rec = sm.tile([1, 512], F32, tag="rec", name="rec")
                        nc.vector.reciprocal_approx_fast(rec[:], dens[hh][:])
                        rb = sm.tile([64, 512], F32, tag="rb", name="rb")
                        nc.gpsimd.partition_broadcast(rb[:], rec[:])
                        nc.vector.tensor_mul(
                            ots[pair][hh * 64:(hh + 1) * 64, :],
                            uns[hh][:], rb[:],
                        )
                ots_prev = ots
            emit_op(NQ - 1, ots_prev, [(pj, "pj"), (px, "x")])

    nc.compile()
    return nc


def _get_nc():
    if "nc" not in _CACHE:
        _CACHE["nc"] = _build()
    return _CACHE["nc"]


def kernel(Q, K, V, Wq, bq, Wk, bk, Wv, bv, Wo, bo):
    nc = _get_nc()
    f = np.float32
    bf = np.float16
    in_maps = []
    for core in range(8):
        b, g = divmod(core, 4)
        sl = slice(g * OL, (g + 1) * OL)
        in_maps.append({
            "qt": np.ascontiguousarray(Q[b].T, dtype=bf),
            "kt": np.ascontiguousarray(K[b].T, dtype=bf),
            "vt": np.ascontiguousarray(V[b].T, dtype=bf),
            "wqt": np.ascontiguousarray(Wq[sl].T, dtype=bf),
            "wkt": np.ascontiguousarray(Wk[sl].T, dtype=bf),
            "wvt": np.ascontiguousarray(Wv[sl].T, dtype=bf),
            "bq2": np.ascontiguousarray(bq[sl].reshape(2, 128, 1), dtype=f),
            "bk2": np.ascontiguousarray(bk[sl].reshape(2, 128, 1), dtype=f),
            "bv1": np.ascontiguousarray(bv[sl].reshape(1, OL), dtype=bf),
            "wot": np.ascontiguousarray(Wo[:, sl].T, dtype=bf),
        })
    res = run_bass_kernel_spmd(nc, in_maps, core_ids=list(range(8)))
    out = np.empty((B, S, D), np.float32)
    for b in range(B):
        acc = res.results[b * 4 + 0]["out_t"].astype(np.float64)
        for g in range(1, 4):
            acc += res.results[b * 4 + g]["out_t"]
        out[b] = (acc.T + bo).astype(np.float32)
    return out


# revision 14
# speedup vs baseline: 1.1150x; 1.1150x over previous
"""Multi-head attention (B=2, S=2048, D=1024, H=16) on 8 Trainium2 NeuronCores.

Sharding: core c -> batch b = c // 4, head-group g = c % 4 (4 heads = 256 proj
dims per core). Each core computes its 4 heads' attention plus the matching
slice of the output projection; the host sums the 4 partial outputs per batch
and adds bo.

Device layouts (matmul operands float32r = fp32 bits at bf16 PE rate):
  qT/kT [o, s]   : proj from host-transposed Q/K (contraction on partitions)
  v     [s, o]   : natural layout + ones column per head (softmax denominator
                   rides along row 64 of the PV matmul output)
  scoresT [k, q] : head pairs row-packed on the PE (base_partition 0/64);
                   both halves of a [128,1024] PSUM tile -> one wide Exp
  outT  [d, q]   : unnormalized; moved off PSUM fast, normalized with
                   reciprocal_approx_fast + GpSimd partition_broadcast
  out_pT [o, q]  : local slice of x @ Wo.T; host transposes + sums + bias
"""

import ml_dtypes
import numpy as np

import concourse.bass as bass
import concourse.mybir as mybir
import concourse.tile as tile
from concourse import bacc
from concourse.bass_utils import run_bass_kernel_spmd

B, S, D, H = 2, 2048, 1024, 16
OL = 256          # local projection dims (4 heads x 64)
NI = D // 128     # contraction chunks for projections
NK = S // 128     # key chunks
NQ = S // 512     # query blocks

_CACHE = {}


def _build():
    DT = mybir.dt.float16
    F32 = mybir.dt.float32
    AF = mybir.ActivationFunctionType

    nc = bacc.Bacc("TRN2", target_bir_lowering=False, debug=False, num_devices=8)

    qt_d = nc.dram_tensor("qt", [D, S], DT, kind="ExternalInput").ap() \
        .rearrange("(c p) s -> c p s", p=128)
    kt_d = nc.dram_tensor("kt", [D, S], DT, kind="ExternalInput").ap() \
        .rearrange("(c p) s -> c p s", p=128)
    vt_d = nc.dram_tensor("vt", [D, S], DT, kind="ExternalInput").ap() \
        .rearrange("(c p) s -> c p s", p=128)
    wq_d = nc.dram_tensor("wqt", [D, OL], DT, kind="ExternalInput").ap() \
        .rearrange("(c p) o -> c p o", p=128)
    wk_d = nc.dram_tensor("wkt", [D, OL], DT, kind="ExternalInput").ap() \
        .rearrange("(c p) o -> c p o", p=128)
    wv_d = nc.dram_tensor("wvt", [D, OL], DT, kind="ExternalInput").ap() \
        .rearrange("(c p) o -> c p o", p=128)
    bq_d = nc.dram_tensor("bq2", [2, 128, 1], F32, kind="ExternalInput").ap()
    bk_d = nc.dram_tensor("bk2", [2, 128, 1], F32, kind="ExternalInput").ap()
    bv_d = nc.dram_tensor("bv1", [1, OL], DT, kind="ExternalInput").ap()
    wo_d = nc.dram_tensor("wot", [OL, D], DT, kind="ExternalInput").ap() \
        .rearrange("(c p) o -> c p o", p=128)
    out_d = nc.dram_tensor("out_t", [D, S], F32, kind="ExternalOutput").ap() \
        .rearrange("(c p) s -> c p s", p=128)

    with tile.TileContext(nc) as tc:
        with (
            tc.tile_pool(name="per", bufs=1) as per,
            tc.tile_pool(name="wp", bufs=1) as wp,
            tc.tile_pool(name="ip", bufs=1) as ip,
            tc.tile_pool(name="pr", bufs=8) as pr,
            tc.tile_pool(name="sm", bufs=2) as sm,
            tc.tile_pool(name="ot", bufs=2) as ot,
            tc.tile_pool(name="osg", bufs=3) as osg,
            tc.tile_pool(name="pj", bufs=2, space="PSUM") as pj,
            tc.tile_pool(name="p1", bufs=2, space="PSUM") as p1,
            tc.tile_pool(name="px", bufs=2, space="PSUM") as px,
        ):
            # --- persistent tiles
            qt_sb = [per.tile([128, S], DT, tag=f"qt{m}", name=f"qt{m}")
                     for m in range(2)]
            kt_sb = [per.tile([128, S], DT, tag=f"kt{m}", name=f"kt{m}")
                     for m in range(2)]
            v_sb = [per.tile([128, 4, 65], DT, tag=f"v{sc}", name=f"v{sc}")
                    for sc in range(NK)]
            wo_sb = [per.tile([128, D], DT, tag=f"wo{c}", name=f"wo{c}")
                     for c in range(2)]
            bq_sb = [per.tile([128, 1], F32, tag=f"bq{m}", name=f"bq{m}")
                     for m in range(2)]
            bk_sb = [per.tile([128, 1], F32, tag=f"bk{m}", name=f"bk{m}")
                     for m in range(2)]
            bv_sb = per.tile([1, OL], DT, tag="bv", name="bv")
            ones_f = per.tile([1, 128], F32, tag="ones_f", name="ones_f")
            vones_f = per.tile([128, 1], F32, tag="vones_f", name="vones_f")
            ones_r = per.tile([1, 128], DT, tag="ones_r", name="ones_r")
            nc.vector.memset(ones_f[:], 1.0)
            nc.vector.memset(vones_f[:], 1.0)
            nc.vector.tensor_copy(ones_r[:], ones_f[:])

            for m in range(2):
                nc.sync.dma_start(bq_sb[m][:], bq_d[m])
                nc.sync.dma_start(bk_sb[m][:], bk_d[m])
            nc.sync.dma_start(bv_sb[:], bv_d)
            for c in range(2):
                nc.sync.dma_start(wo_sb[c][:], wo_d[c])

            # --- input loads, all on Sync HWDGE, in consumption order.
            # Per-queue DMA is ~25 GB/s and a trigger costs ~0.6us, so
            # 128KB column-quarters spread across the 8 queues are optimal.
            # Only kt + qt[s0] load up front; vt streams into qb0's kc loop
            # and qt[s1..3] stream into the preceding query block.
            def load_w(w_dr, key, eng):
                ws = []
                for i in range(NI):
                    w = wp.tile([128, OL], DT, tag=f"w{key}{i}",
                                name=f"w{key}{i}")
                    eng.dma_start(w[:], w_dr[i])
                    ws.append(w)
                return ws

            def alloc_a(key):
                return [ip.tile([128, S], DT, tag=f"a{key}{i}",
                                name=f"a{key}{i}") for i in range(NI)]

            def issue_a(as_, a_dr, csl):
                for i in range(NI):
                    nc.sync.dma_start(as_[i][:, csl], a_dr[i][:, csl])

            ws_k = load_w(wk_d, "k", nc.sync)
            as_k = alloc_a("k")
            for s4 in range(4):
                issue_a(as_k, kt_d, slice(s4 * 512, (s4 + 1) * 512))
            ws_q = load_w(wq_d, "q", nc.sync)
            as_q = alloc_a("q")
            issue_a(as_q, qt_d, slice(0, 512))
            ws_v = load_w(wv_d, "v", nc.sync)
            as_v = alloc_a("v")
            issue_a(as_v, vt_d, slice(0, 512))

            def q_chain(ws, as_, bias_sb, dst_sb, m, s, on_act):
                acc = pj.tile([128, 512], F32, tag="pj", name="pj")
                for i in range(NI):
                    nc.tensor.matmul(
                        acc[:],
                        ws[i][:, m * 128:(m + 1) * 128],
                        as_[i][:, s * 512:(s + 1) * 512],
                        start=(i == 0),
                        stop=(i == NI - 1),
                    )
                dst = dst_sb[m][:, s * 512:(s + 1) * 512]
                if on_act:
                    nc.scalar.activation(
                        dst, acc[:], AF.Identity, bias=bias_sb[m][:])
                else:
                    nc.vector.tensor_scalar_add(dst, acc[:], bias_sb[m][:])

            def v_chain(sc):
                acc = pj.tile([128, OL], F32, tag="pj", name="pj")
                for i in range(NI):
                    nc.tensor.matmul(
                        acc[:],
                        as_v[i][:, sc * 128:(sc + 1) * 128],
                        ws_v[i][:],
                        start=(i == 0),
                        stop=False,
                    )
                nc.tensor.matmul(
                    acc[:], ones_r[:], bv_sb[:], start=False, stop=True
                )
                for h in range(4):
                    nc.vector.tensor_copy(
                        v_sb[sc][:, h, 0:64],
                        acc[:, h * 64:(h + 1) * 64],
                    )

            for sc in range(NK):
                nc.vector.tensor_copy(
                    v_sb[sc][:, :, 64:65],
                    vones_f[:].to_broadcast((128, 4, 1)),
                )
            # prefix chains: all of kT, then qT column s0 (ACT is idle here)
            for sg in range(4):
                for m in range(2):
                    q_chain(ws_k, as_k, bk_sb, kt_sb, m, sg, True)
            for m in range(2):
                q_chain(ws_q, as_q, bq_sb, qt_sb, m, 0, True)

            # --- attention + output projection, per query block
            def emit_op(qb, ots_prev, oc, pool, tg):
                osl = slice(oc * 128, (oc + 1) * 128)
                pso = pool.tile([128, 512], F32, tag=tg, name="pso")
                for c in range(2):
                    nc.tensor.matmul(
                        pso[:], wo_sb[c][:, osl], ots_prev[c][:],
                        start=(c == 0), stop=(c == 1),
                    )
                st = osg.tile([128, 512], F32, tag="st", name="st")
                nc.vector.tensor_copy(st[:], pso[:])
                nc.sync.dma_start(
                    out_d[oc][:, qb * 512:(qb + 1) * 512], st[:])

            ots_prev = None
            for qb in range(NQ):
                qsl = slice(qb * 512, (qb + 1) * 512)
                ots = [ot.tile([128, 512], DT, tag=f"c{c}", name=f"otc{c}")
                       for c in range(2)]
                for pair in range(2):
                    acc = [px.tile([65, 512], F32, tag="x", name="acc")
                           for _ in range(2)]
                    pend = []
                    op_iter = None
                    if pair == 0 and ots_prev is not None:
                        op_iter = iter(range(8))
                    for kc in range(NK):
                        if qb == 0 and pair == 0:
                            # stream vt quarters + v chains just in time
                            if kc < 12 and kc % 2 == 0:
                                j = kc // 4 + 1
                                half = (kc % 4) // 2
                                csl = slice(j * 512, (j + 1) * 512)
                                for i in range(NI // 2 * half,
                                               NI // 2 * (half + 1)):
                                    nc.sync.dma_start(
                                        as_v[i][:, csl], vt_d[i][:, csl])
                            v_chain(kc)
                        if pair == 1 and qb < NQ - 1:
                            # stream qt[s=qb+1] + its projection chains
                            if kc == 0:
                                issue_a(as_q, qt_d,
                                        slice((qb + 1) * 512, (qb + 2) * 512))
                            elif kc == 6 or kc == 11:
                                q_chain(ws_q, as_q, bq_sb, qt_sb,
                                        kc == 11, qb + 1, False)
                        ksl = slice(kc * 128, (kc + 1) * 128)
                        ps1 = p1.tile([128, 1024], F32, tag="s", name="s")
                        for hh in range(2):
                            psl = slice(hh * 64, (hh + 1) * 64)
                            nc.tensor.matmul(
                                ps1[:, hh * 512:(hh + 1) * 512],
                                kt_sb[pair][psl, ksl],
                                qt_sb[pair][psl, qsl],
                                start=True, stop=True,
                            )
                        prob = pr.tile([128, 1024], DT, tag="p", name="p")
                        nc.scalar.activation(
                            prob[:], ps1[:], AF.Exp, scale=0.125
                        )
                        pend.append((kc, prob))
                        if len(pend) > 2:
                            pkc, pprob = pend.pop(0)
                            for hh in range(2):
                                nc.tensor.matmul(
                                    acc[hh][:], v_sb[pkc][:, pair * 2 + hh, :],
                                    pprob[:, hh * 512:(hh + 1) * 512],
                                    start=(pkc == 0), stop=(pkc == NK - 1),
                                )
                        if op_iter is not None and kc % 2 == 1:
                            oc = next(op_iter, None)
                            if oc is not None:
                                emit_op(qb - 1, ots_prev, oc, pj, "pj")
                    for pkc, pprob in pend:
                        for hh in range(2):
                            nc.tensor.matmul(
                                acc[hh][:], v_sb[pkc][:, pair * 2 + hh, :],
                                pprob[:, hh * 512:(hh + 1) * 512],
                                start=(pkc == 0), stop=(pkc == NK - 1),
                            )
                    # normalize off-bank: free both acc banks first
                    uns, dens = [], []
                    for hh in range(2):
                        un = sm.tile([64, 512], F32, tag=f"un{hh}",
                                     name=f"un{hh}")
                        nc.vector.tensor_copy(un[:], acc[hh][0:64, :])
                        den = sm.tile([1, 512], F32, tag=f"den{hh}",
                                      name=f"den{hh}")
                        nc.vector.tensor_copy(den[:], acc[hh][64:65, :])
                        uns.append(un)
                        dens.append(den)
                    for hh in range(2):
                        rec = sm.tile([1, 512], F32, tag="rec", name="rec")
                        nc.vector.reciprocal_approx_fast(rec[:], dens[hh][:])
                        rb = sm.tile([64, 512], F32, tag="rb", name="rb")
                        nc.gpsimd.partition_broadcast(rb[:], rec[:])
                        nc.vector.tensor_mul(
                            ots[pair][hh * 64:(hh + 1) * 64, :],
                            uns[hh][:], rb[:],
                        )
                ots_prev = ots
            for oc in range(8):
                emit_op(NQ - 1, ots_prev, oc, (pj, px)[oc % 2],
                        ("pj", "x")[oc % 2])

    nc.compile()
    return nc


def _get_nc():
    if "nc" not in _CACHE:
        _CACHE["nc"] = _build()
    return _CACHE["nc"]


def kernel(Q, K, V, Wq, bq, Wk, bk, Wv, bv, Wo, bo):
    nc = _get_nc()
    f = np.float32
    bf = np.float16
    in_maps = []
    for core in range(8):
        b, g = divmod(core, 4)
        sl = slice(g * OL, (g + 1) * OL)
        in_maps.append({
            "qt": np.ascontiguousarray(Q[b].T, dtype=bf),
            "kt": np.ascontiguousarray(K[b].T, dtype=bf),
            "vt": np.ascontiguousarray(V[b].T, dtype=bf),
            "wqt": np.ascontiguousarray(Wq[sl].T, dtype=bf),
            "wkt": np.ascontiguousarray(Wk[sl].T, dtype=bf),
            "wvt": np.ascontiguousarray(Wv[sl].T, dtype=bf),
            "bq2": np.ascontiguousarray(bq[sl].reshape(2, 128, 1), dtype=f),
            "bk2": np.ascontiguousarray(bk[sl].reshape(2, 128, 1), dtype=f),
            "bv1": np.ascontiguousarray(bv[sl].reshape(1, OL), dtype=bf),
            "wot": np.ascontiguousarray(Wo[:, sl].T, dtype=bf),
        })
    res = run_bass_kernel_spmd(nc, in_maps, core_ids=list(range(8)))
    out = np.empty((B, S, D), np.float32)
    for b in range(B):
        acc = res.results[b * 4 + 0]["out_t"].astype(np.float64)
        for g in range(1, 4):
            acc += res.results[b * 4 + g]["out_t"]
        out[b] = (acc.T + bo).astype(np.float32)
    return out
